# revision 4
# baseline (speedup 1.0000x reference)
"""Bass/Trainium2 kernel for nn_Encoder_32452772888844.

64 independent 2-layer LSTM(256) encoders + per-group Linear(256,256),
then shared heads:
  lin1  = fc @ W1.T + b1
  delta = softmax(lin1, axis=0)   (over the 64 groups)
  beta  = softplus(fc @ W2.T + b2)
  gamma = lin1 @ Wd.T + bd
Sharding: pure group parallelism — 8 groups per NeuronCore; each core
computes everything for its groups including exp(lin1); the softmax
normalization (a sum over the 64-group axis) is applied on the host.

Device-side math formulation (per core):
  - All matmuls are weight-stationary: lhsT tiles [K=128, M=128] are
    (transposed) weight blocks, the moving operand is the activation
    vector/sequence. Gate results land in PSUM with the hidden dim on
    partitions, which makes the LSTM cell elementwise work efficient.
  - Weights are cast to bf16 on the host (PSUM accumulates in fp32).
  - x-projections for all 10 timesteps are batched; the h-recurrence
    runs 16 LDW+MM pairs (N=1) per group-step-layer.
"""

import numpy as np
import ml_dtypes

T = 10
IN = 256
H = 256
G = 64
NCORES = 8
GPC = G // NCORES  # groups per core
A = 2   # 128-halves of 256
C8 = 8  # 128-chunks of 1024
C2 = 2  # 128-chunks of 256

BF16 = ml_dtypes.bfloat16

_COMPILED = {}


def _build_nc():
    import concourse.tile as tile
    from concourse import bacc, mybir

    f32 = mybir.dt.float32
    bf16 = mybir.dt.bfloat16
    Sig = mybir.ActivationFunctionType.Sigmoid
    Tanh = mybir.ActivationFunctionType.Tanh
    Exp = mybir.ActivationFunctionType.Exp
    Ln = mybir.ActivationFunctionType.Ln

    nc = bacc.Bacc(None, target_bir_lowering=False)

    # ---- DRAM parameters (per-core shards, host-prepared layouts) ----
    d_xT = nc.dram_tensor("xT", [128, GPC, A, T], bf16, kind="ExternalInput")
    d_wih = [
        nc.dram_tensor("wih0", [128, GPC, A, C8, 128], bf16, kind="ExternalInput"),
        nc.dram_tensor("wih1", [128, GPC, A, C8, 128], bf16, kind="ExternalInput"),
    ]
    d_whh = [
        nc.dram_tensor("whh0", [128, GPC, A, C8, 128], bf16, kind="ExternalInput"),
        nc.dram_tensor("whh1", [128, GPC, A, C8, 128], bf16, kind="ExternalInput"),
    ]
    d_wlin = nc.dram_tensor("wlin", [128, GPC, A, C2, 128], bf16, kind="ExternalInput")
    d_w1t = nc.dram_tensor("w1t", [128, A, C2, 128], bf16, kind="ExternalInput")
    d_w2t = nc.dram_tensor("w2t", [128, A, C2, 128], bf16, kind="ExternalInput")
    d_wdt = nc.dram_tensor("wdt", [128, A], bf16, kind="ExternalInput")
    d_b = [
        nc.dram_tensor("b0", [128, GPC, C8], f32, kind="ExternalInput"),
        nc.dram_tensor("b1c", [128, GPC, C8], f32, kind="ExternalInput"),
    ]
    d_blin = nc.dram_tensor("blin", [128, GPC, C2], f32, kind="ExternalInput")
    d_b1 = nc.dram_tensor("b1", [128, C2], f32, kind="ExternalInput")
    d_b2 = nc.dram_tensor("b2", [128, C2], f32, kind="ExternalInput")
    d_bd = nc.dram_tensor("bd", [1, 1], f32, kind="ExternalInput")

    d_lin1 = nc.dram_tensor("lin1o", [128, C2, GPC, T], f32, kind="ExternalOutput")
    d_expl = nc.dram_tensor("explo", [128, C2, GPC, T], f32, kind="ExternalOutput")
    d_beta = nc.dram_tensor("betao", [128, C2, GPC, T], f32, kind="ExternalOutput")
    d_gamma = nc.dram_tensor("gammao", [1, GPC, T], f32, kind="ExternalOutput")
    d_hn = nc.dram_tensor("hno", [128, 2, GPC, A], f32, kind="ExternalOutput")
    d_cn = nc.dram_tensor("cno", [128, 2, GPC, A], f32, kind="ExternalOutput")

    with tile.TileContext(nc) as tc:
        with (
            tc.tile_pool(name="wpool", bufs=1) as wpool,
            tc.tile_pool(name="apool", bufs=1) as apool,
            tc.tile_pool(name="tpool", bufs=1) as tpool,
            tc.tile_pool(name="pspool", bufs=1, space="PSUM") as pspool,
        ):
            # ---- weight/bias/data loads ----
            # Per-group weight tiles so compute on group g only waits on
            # g's own DMAs (pipelining DMA with compute).
            w_ih = [[None] * GPC for _ in range(2)]
            w_hh = [[None] * GPC for _ in range(2)]
            w_lin = [None] * GPC
            for g in range(GPC):
                for l in range(2):
                    w_ih[l][g] = wpool.tile(
                        [128, A, C8, 128], bf16, tag=f"wih{l}_{g}", name=f"wih{l}_{g}"
                    )
                    nc.sync.dma_start(w_ih[l][g][:], d_wih[l][:, g])
                    w_hh[l][g] = wpool.tile(
                        [128, A, C8, 128], bf16, tag=f"whh{l}_{g}", name=f"whh{l}_{g}"
                    )
                    nc.sync.dma_start(w_hh[l][g][:], d_whh[l][:, g])
                w_lin[g] = wpool.tile(
                    [128, A, C2, 128], bf16, tag=f"wlin_{g}", name=f"wlin_{g}"
                )
                nc.sync.dma_start(w_lin[g][:], d_wlin[:, g])

            s_xT = apool.tile([128, GPC, A, T], bf16, tag="xT", name="s_xT")
            nc.sync.dma_start(s_xT[:], d_xT[:])
            s_w1t = apool.tile([128, A, C2, 128], bf16, tag="w1t", name="s_w1t")
            nc.sync.dma_start(s_w1t[:], d_w1t[:])
            s_w2t = apool.tile([128, A, C2, 128], bf16, tag="w2t", name="s_w2t")
            nc.sync.dma_start(s_w2t[:], d_w2t[:])
            s_wdt = apool.tile([128, A], bf16, tag="wdt", name="s_wdt")
            nc.sync.dma_start(s_wdt[:], d_wdt[:])
            s_b = []
            for l in range(2):
                t_ = apool.tile([128, GPC, C8], f32, tag=f"b{l}", name=f"s_b{l}")
                nc.sync.dma_start(t_[:], d_b[l][:])
                s_b.append(t_)
            s_blin = apool.tile([128, GPC, C2], f32, tag="blin", name="s_blin")
            nc.sync.dma_start(s_blin[:], d_blin[:])
            s_b1 = apool.tile([128, C2], f32, tag="b1", name="s_b1")
            nc.sync.dma_start(s_b1[:], d_b1[:])
            s_b2 = apool.tile([128, C2], f32, tag="b2", name="s_b2")
            nc.sync.dma_start(s_b2[:], d_b2[:])
            s_bd = apool.tile([1, 1], f32, tag="bd", name="s_bd")
            nc.sync.dma_start(s_bd[:], d_bd[:])

            # ---- persistent activation buffers ----
            hbuf = [
                apool.tile([128, GPC, A, T], bf16, tag=f"hbuf{l}", name=f"hbuf{l}")
                for l in range(2)
            ]
            fcbf = apool.tile([128, C2, GPC, T], bf16, tag="fcbf", name="fcbf")
            lin1bf = apool.tile([128, C2, GPC, T], bf16, tag="lin1bf", name="lin1bf")
            s_lin1 = apool.tile([128, C2, GPC, T], f32, tag="lin1", name="s_lin1")
            s_expl = apool.tile([128, C2, GPC, T], f32, tag="expl", name="s_expl")
            s_beta = apool.tile([128, C2, GPC, T], f32, tag="beta", name="s_beta")
            s_gamma = apool.tile([1, GPC, T], f32, tag="gamma", name="s_gamma")
            s_hn = apool.tile([128, 2, GPC, A], f32, tag="hn", name="s_hn")
            s_cn = apool.tile([128, 2, GPC, A], f32, tag="cn", name="s_cn")

            NH = 2           # group-halves for elementwise batching
            GH = GPC // NH   # groups per half

            # ================= the two LSTM layers =================
            for l in range(2):
                # --- x-projection for all groups, all timesteps ---
                # xp[p, t, g, c] = (W_ih x_t)[128c+p] + (bih+bhh)[128c+p]
                xp = apool.tile(
                    [128, T, GPC, C8], f32, tag="xp", bufs=2, name=f"xp{l}"
                )
                for g in range(GPC):
                    for c in range(C8):
                        ps = pspool.tile(
                            [128, T], f32, tag="xp_ps", bufs=2, name=f"xps{l}_{g}_{c}"
                        )
                        for a in range(A):
                            if l == 0:
                                rhs = s_xT[:, g, a, :]
                            else:
                                rhs = hbuf[0][:, g, a, :]
                            nc.tensor.matmul(
                                ps[:],
                                w_ih[l][g][:, a, c, :],
                                rhs,
                                start=(a == 0),
                                stop=(a == 1),
                            )
                        nc.vector.tensor_scalar_add(
                            xp[:, :, g, c], ps[:], s_b[l][:, g, c : c + 1]
                        )

                # --- recurrence ---
                cst = [
                    apool.tile(
                        [128, GH, A], f32, tag=f"cst{l}_{h}", name=f"cst{l}_{h}"
                    )
                    for h in range(NH)
                ]
                for t in range(T):
                    pss = []
                    for h in range(NH):
                        g0 = h * GH
                        if t > 0:
                            ps = pspool.tile(
                                [128, GH, C8],
                                f32,
                                tag="gate_ps",
                                bufs=3,
                                name=f"gps{l}_{t}_{h}",
                            )
                            for gi in range(GH):
                                g = g0 + gi
                                for c in range(C8):
                                    for a in range(A):
                                        nc.tensor.matmul(
                                            ps[:, gi, c : c + 1],
                                            w_hh[l][g][:, a, c, :],
                                            hbuf[l][:, g, a, t - 1 : t],
                                            start=(a == 0),
                                            stop=(a == 1),
                                        )
                            pss.append(ps)
                        else:
                            pss.append(None)

                    for h in range(NH):
                        g0 = h * GH
                        ps = pss[h]
                        if t > 0:
                            gb = tpool.tile(
                                [128, GH, C8], f32, tag="gb", bufs=3, name=f"gb{l}{t}{h}"
                            )
                            nc.vector.tensor_add(
                                gb[:], ps[:], xp[:, t, g0 : g0 + GH, :]
                            )
                            src = gb
                        else:
                            src = xp[:, 0, g0 : g0 + GH, :]
                        sif = tpool.tile(
                            [128, GH, 4], f32, tag="sif", bufs=3, name=f"sif{l}{t}{h}"
                        )
                        nc.scalar.activation(sif[:], src[:, :, 0:4], Sig)
                        tg = tpool.tile(
                            [128, GH, A], f32, tag="tg", bufs=3, name=f"tg{l}{t}{h}"
                        )
                        nc.scalar.activation(tg[:], src[:, :, 4:6], Tanh)
                        so = tpool.tile(
                            [128, GH, A], f32, tag="so", bufs=3, name=f"so{l}{t}{h}"
                        )
                        nc.scalar.activation(so[:], src[:, :, 6:8], Sig)

                        if t == 0:
                            # c = i * g
                            nc.vector.tensor_mul(cst[h][:], sif[:, :, 0:2], tg[:])
                        else:
                            fmul = tpool.tile(
                                [128, GH, A], f32, tag="fmul", bufs=3,
                                name=f"fm{l}{t}{h}",
                            )
                            nc.vector.tensor_mul(fmul[:], sif[:, :, 2:4], cst[h][:])
                            ig = tpool.tile(
                                [128, GH, A], f32, tag="ig", bufs=3, name=f"ig{l}{t}{h}"
                            )
                            nc.vector.tensor_mul(ig[:], sif[:, :, 0:2], tg[:])
                            nc.vector.tensor_add(cst[h][:], fmul[:], ig[:])
                        tch = tpool.tile(
                            [128, GH, A], f32, tag="tch", bufs=3, name=f"tc{l}{t}{h}"
                        )
                        nc.scalar.activation(tch[:], cst[h][:], Tanh)
                        # h_t (bf16) into the sequence buffer
                        nc.vector.tensor_mul(
                            hbuf[l][:, g0 : g0 + GH, :, t], so[:], tch[:]
                        )
                        if t == T - 1:
                            nc.vector.tensor_mul(
                                s_hn[:, l, g0 : g0 + GH, :], so[:], tch[:]
                            )
                            nc.vector.tensor_copy(
                                s_cn[:, l, g0 : g0 + GH, :], cst[h][:]
                            )

            # ================= per-group Linear =================
            for g in range(GPC):
                ps = pspool.tile(
                    [128, C2, T], f32, tag="fc_ps", bufs=1, name=f"fcps{g}"
                )
                for c2 in range(C2):
                    for a in range(A):
                        nc.tensor.matmul(
                            ps[:, c2, :],
                            w_lin[g][:, a, c2, :],
                            hbuf[1][:, g, a, :],
                            start=(a == 0),
                            stop=(a == 1),
                        )
                for c2 in range(C2):
                    nc.vector.tensor_scalar_add(
                        fcbf[:, c2, g, :], ps[:, c2, :], s_blin[:, g, c2 : c2 + 1]
                    )

            # ================= shared heads (batched over groups) =========
            # softplus = ln(1 + exp(x)): all Exp ACTs first, then all Ln
            # ACTs, so the engine switches activation tables only once.
            betaexp = [None, None]
            for c2 in range(C2):
                psA = pspool.tile(
                    [128, GPC * T], f32, tag="head", bufs=2, name=f"psA{c2}"
                )
                for a in range(A):
                    nc.tensor.matmul(
                        psA[:],
                        s_w1t[:, a, c2, :],
                        fcbf[:, a].rearrange("p g t -> p (g t)"),
                        start=(a == 0),
                        stop=(a == 1),
                    )
                nc.vector.tensor_scalar_add(
                    s_lin1[:, c2].rearrange("p g t -> p (g t)"),
                    psA[:],
                    s_b1[:, c2 : c2 + 1],
                )
                nc.vector.tensor_scalar_add(
                    lin1bf[:, c2].rearrange("p g t -> p (g t)"),
                    psA[:],
                    s_b1[:, c2 : c2 + 1],
                )
                nc.scalar.activation(
                    s_expl[:, c2].rearrange("p g t -> p (g t)"),
                    psA[:],
                    Exp,
                    bias=s_b1[:, c2 : c2 + 1],
                )
                psB = pspool.tile(
                    [128, GPC * T], f32, tag="head", bufs=2, name=f"psB{c2}"
                )
                for a in range(A):
                    nc.tensor.matmul(
                        psB[:],
                        s_w2t[:, a, c2, :],
                        fcbf[:, a].rearrange("p g t -> p (g t)"),
                        start=(a == 0),
                        stop=(a == 1),
                    )
                be = tpool.tile(
                    [128, GPC * T], f32, tag=f"betaexp{c2}", name=f"betaexp{c2}"
                )
                nc.scalar.activation(be[:], psB[:], Exp, bias=s_b2[:, c2 : c2 + 1])
                betaexp[c2] = be
            for c2 in range(C2):
                nc.scalar.activation(
                    s_beta[:, c2].rearrange("p g t -> p (g t)"),
                    betaexp[c2][:],
                    Ln,
                    bias=1.0,
                )

            psG = pspool.tile([1, GPC * T], f32, tag="head", bufs=2, name="psG")
            for a in range(A):
                nc.tensor.matmul(
                    psG[:],
                    s_wdt[:, a : a + 1],
                    lin1bf[:, a].rearrange("p g t -> p (g t)"),
                    start=(a == 0),
                    stop=(a == 1),
                )
            nc.vector.tensor_scalar_add(
                s_gamma.rearrange("p g t -> p (g t)"), psG[:], s_bd[:, 0:1]
            )

            # ---- outputs ----
            nc.sync.dma_start(d_lin1[:], s_lin1[:])
            nc.sync.dma_start(d_expl[:], s_expl[:])
            nc.sync.dma_start(d_beta[:], s_beta[:])
            nc.sync.dma_start(d_gamma[:], s_gamma[:])
            nc.sync.dma_start(d_hn[:], s_hn[:])
            nc.sync.dma_start(d_cn[:], s_cn[:])

    nc.compile()
    return nc


def _prep_core_inputs(core, data, Wih0, Whh0, bih0, bhh0, Wih1, Whh1, bih1, bhh1,
                      Wlin, blin, W1, b1, W2, b2, Wd, bd):
    """Host-side shard + retile + bf16 cast for one core."""
    lo, hi = core * GPC, (core + 1) * GPC

    def wtiles(W, cdim):
        # W: [GPC, cdim*128, 256] -> [128(q), GPC, A, cdim, 128(m)]
        arr = W.reshape(GPC, cdim, 128, A, 128)  # [g, c, m, a, q]
        return np.ascontiguousarray(arr.transpose(4, 0, 3, 1, 2)).astype(BF16)

    def btiles(b, cdim):
        # b: [GPC, cdim*128] -> [128(p), GPC, cdim]
        return np.ascontiguousarray(
            b.reshape(GPC, cdim, 128).transpose(2, 0, 1)
        ).astype(np.float32)

    d = {}
    x = data[:, lo:hi, :]  # [T, GPC, 256]
    d["xT"] = np.ascontiguousarray(
        x.reshape(T, GPC, A, 128).transpose(3, 1, 2, 0)
    ).astype(BF16)
    d["wih0"] = wtiles(Wih0[lo:hi], C8)
    d["whh0"] = wtiles(Whh0[lo:hi], C8)
    d["wih1"] = wtiles(Wih1[lo:hi], C8)
    d["whh1"] = wtiles(Whh1[lo:hi], C8)
    d["wlin"] = wtiles(Wlin[lo:hi], C2)
    d["b0"] = btiles(bih0[lo:hi] + bhh0[lo:hi], C8)
    d["b1c"] = btiles(bih1[lo:hi] + bhh1[lo:hi], C8)
    d["blin"] = btiles(blin[lo:hi], C2)
    # shared weights: [C2*128, 256] -> [128(q), A, C2, 128(m)]
    for name, W in (("w1t", W1), ("w2t", W2)):
        arr = W.reshape(C2, 128, A, 128)  # [c2, m, a, q]
        d[name] = np.ascontiguousarray(arr.transpose(3, 2, 0, 1)).astype(BF16)
    d["wdt"] = np.ascontiguousarray(Wd.reshape(A, 128).T).astype(BF16)
    d["b1"] = np.ascontiguousarray(b1.reshape(C2, 128).T).astype(np.float32)
    d["b2"] = np.ascontiguousarray(b2.reshape(C2, 128).T).astype(np.float32)
    d["bd"] = bd.reshape(1, 1).astype(np.float32)
    return d


def _get_compiled():
    if "nc" not in _COMPILED:
        _COMPILED["nc"] = _build_nc()
    return _COMPILED["nc"]


def run_device(in_maps, trace=False, tmpdir=None):
    from concourse import bass_utils

    nc = _get_compiled()
    kw = {}
    if trace:
        kw = dict(trace=True, tmpdir=tmpdir)
    res = bass_utils.run_bass_kernel_spmd(
        nc, in_maps, core_ids=list(range(NCORES)), **kw
    )
    return res


def assemble(results):
    """Per-core device outputs -> full reference-shaped outputs."""
    lin1 = np.empty((G, T, H), np.float32)
    expl = np.empty((G, T, H), np.float32)
    beta = np.empty((G, T, H), np.float32)
    gamma = np.empty((G, T, 1), np.float32)
    hN = np.empty((G, 2, H), np.float32)
    cN = np.empty((G, 2, H), np.float32)
    for core in range(NCORES):
        r = results[core]
        lo = core * GPC
        # [128(p), C2, GPC, T] -> [g, t, c2*128+p]
        for name, dst in (("lin1o", lin1), ("explo", expl), ("betao", beta)):
            v = r[name].reshape(128, C2, GPC, T)
            dst[lo : lo + GPC] = v.transpose(2, 3, 1, 0).reshape(GPC, T, H)
        gamma[lo : lo + GPC] = r["gammao"].reshape(GPC, T, 1)
        # [128(p), l, g, a] -> [g, l, a*128+p]
        for name, dst in (("hno", hN), ("cno", cN)):
            v = r[name].reshape(128, 2, GPC, A)
            dst[lo : lo + GPC] = v.transpose(2, 1, 3, 0).reshape(GPC, 2, H)
    delta = expl / expl.sum(axis=0, keepdims=True)
    return gamma, beta, delta, hN, cN


def kernel(**inputs):
    in_maps = [_prep_core_inputs(c, **inputs) for c in range(NCORES)]
    res = run_device(in_maps)
    return assemble(res.results)


# revision 10
# speedup vs baseline: 1.2092x; 1.2092x over previous
"""Bass/Trainium2 kernel for nn_Encoder_32452772888844.

64 independent 2-layer LSTM(256) encoders + per-group Linear(256,256),
then shared heads:
  lin1  = fc @ W1.T + b1
  delta = softmax(lin1, axis=0)   (over the 64 groups)
  beta  = softplus(fc @ W2.T + b2)
  gamma = lin1 @ Wd.T + bd
Sharding: pure group parallelism — 8 groups per NeuronCore; each core
computes everything for its groups including exp(lin1); the softmax
normalization (a sum over the 64-group axis) is applied on the host.

Device-side math formulation (per core):
  - All matmuls are weight-stationary: lhsT tiles [K=128, M=128] are
    (transposed) weight blocks, the moving operand is the activation
    vector/sequence. Gate results land in PSUM with the hidden dim on
    partitions, which makes the LSTM cell elementwise work efficient.
  - Weights are cast to bf16 on the host (PSUM accumulates in fp32).
  - x-projections for all 10 timesteps are batched; the h-recurrence
    runs 16 LDW+MM pairs (N=1) per group-step-layer.
"""

import numpy as np
import ml_dtypes

T = 10
IN = 256
H = 256
G = 64
NCORES = 8
GPC = G // NCORES  # groups per core
A = 2   # 128-halves of 256
C8 = 8  # 128-chunks of 1024
C2 = 2  # 128-chunks of 256

BF16 = ml_dtypes.bfloat16

_COMPILED = {}


def _build_nc():
    import concourse.tile as tile
    from concourse import bacc, mybir

    f32 = mybir.dt.float32
    bf16 = mybir.dt.bfloat16
    Sig = mybir.ActivationFunctionType.Sigmoid
    Tanh = mybir.ActivationFunctionType.Tanh
    Exp = mybir.ActivationFunctionType.Exp
    Ln = mybir.ActivationFunctionType.Ln

    nc = bacc.Bacc(None, target_bir_lowering=False)

    # ---- DRAM parameters (per-core shards, host-prepared layouts) ----
    d_xT = nc.dram_tensor("xT", [128, GPC, A, T], bf16, kind="ExternalInput")
    d_wih = [
        nc.dram_tensor("wih0", [128, GPC, A, C8, 128], bf16, kind="ExternalInput"),
        nc.dram_tensor("wih1", [128, GPC, A, C8, 128], bf16, kind="ExternalInput"),
    ]
    d_whh = [
        nc.dram_tensor("whh0", [128, GPC, A, C8, 128], bf16, kind="ExternalInput"),
        nc.dram_tensor("whh1", [128, GPC, A, C8, 128], bf16, kind="ExternalInput"),
    ]
    d_wlin = nc.dram_tensor("wlin", [128, GPC, A, C2, 128], bf16, kind="ExternalInput")
    d_w1t = nc.dram_tensor("w1t", [128, A, C2, 128], bf16, kind="ExternalInput")
    d_w2t = nc.dram_tensor("w2t", [128, A, C2, 128], bf16, kind="ExternalInput")
    d_wdt = nc.dram_tensor("wdt", [128, A], bf16, kind="ExternalInput")
    d_b = [
        nc.dram_tensor("b0", [128, GPC, C8], f32, kind="ExternalInput"),
        nc.dram_tensor("b1c", [128, GPC, C8], f32, kind="ExternalInput"),
    ]
    d_blin = nc.dram_tensor("blin", [128, GPC, C2], f32, kind="ExternalInput")
    d_b1 = nc.dram_tensor("b1", [128, C2], f32, kind="ExternalInput")
    d_b2 = nc.dram_tensor("b2", [128, C2], f32, kind="ExternalInput")
    d_bd = nc.dram_tensor("bd", [1, 1], f32, kind="ExternalInput")

    d_lin1 = nc.dram_tensor("lin1o", [128, C2, GPC, T], f32, kind="ExternalOutput")
    d_expl = nc.dram_tensor("explo", [128, C2, GPC, T], f32, kind="ExternalOutput")
    d_beta = nc.dram_tensor("betao", [128, C2, GPC, T], f32, kind="ExternalOutput")
    d_gamma = nc.dram_tensor("gammao", [1, GPC, T], f32, kind="ExternalOutput")
    d_hn = nc.dram_tensor("hno", [128, 2, GPC, A], f32, kind="ExternalOutput")
    d_cn = nc.dram_tensor("cno", [128, 2, GPC, A], f32, kind="ExternalOutput")

    with tile.TileContext(nc) as tc:
        with (
            tc.tile_pool(name="wpool", bufs=1) as wpool,
            tc.tile_pool(name="apool", bufs=1) as apool,
            tc.tile_pool(name="tpool", bufs=1) as tpool,
            tc.tile_pool(name="pspool", bufs=1, space="PSUM") as pspool,
        ):
            # ---- weight/bias/data loads ----
            # DMAs are emitted in compute-consumption order so the PE can
            # start as soon as xT + group 0's layer-0 weights land:
            # small tensors first, then per-group (wih0,whh0) pairs, then
            # layer 1, then wlin.
            s_xT = apool.tile([128, GPC, A, T], bf16, tag="xT", name="s_xT")
            nc.sync.dma_start(s_xT[:], d_xT[:])
            s_b = []
            for l in range(2):
                t_ = apool.tile([128, GPC, C8], f32, tag=f"b{l}", name=f"s_b{l}")
                nc.sync.dma_start(t_[:], d_b[l][:])
                s_b.append(t_)
            s_blin = apool.tile([128, GPC, C2], f32, tag="blin", name="s_blin")
            nc.sync.dma_start(s_blin[:], d_blin[:])
            s_b1 = apool.tile([128, C2], f32, tag="b1", name="s_b1")
            nc.sync.dma_start(s_b1[:], d_b1[:])
            s_b2 = apool.tile([128, C2], f32, tag="b2", name="s_b2")
            nc.sync.dma_start(s_b2[:], d_b2[:])
            s_bd = apool.tile([1, 1], f32, tag="bd", name="s_bd")
            nc.sync.dma_start(s_bd[:], d_bd[:])
            s_w1t = apool.tile([128, A, C2, 128], bf16, tag="w1t", name="s_w1t")
            nc.sync.dma_start(s_w1t[:], d_w1t[:])
            s_w2t = apool.tile([128, A, C2, 128], bf16, tag="w2t", name="s_w2t")
            nc.sync.dma_start(s_w2t[:], d_w2t[:])
            s_wdt = apool.tile([128, A], bf16, tag="wdt", name="s_wdt")
            nc.sync.dma_start(s_wdt[:], d_wdt[:])

            w_ih = [[None] * GPC for _ in range(2)]
            w_hh = [[None] * GPC for _ in range(2)]
            w_lin = [None] * GPC
            for l in range(2):
                for g in range(GPC):
                    w_ih[l][g] = wpool.tile(
                        [128, A, C8, 128], bf16, tag=f"wih{l}_{g}", name=f"wih{l}_{g}"
                    )
                    nc.sync.dma_start(w_ih[l][g][:], d_wih[l][:, g])
                    w_hh[l][g] = wpool.tile(
                        [128, A, C8, 128], bf16, tag=f"whh{l}_{g}", name=f"whh{l}_{g}"
                    )
                    nc.sync.dma_start(w_hh[l][g][:], d_whh[l][:, g])
            for g in range(GPC):
                w_lin[g] = wpool.tile(
                    [128, A, C2, 128], bf16, tag=f"wlin_{g}", name=f"wlin_{g}"
                )
                nc.sync.dma_start(w_lin[g][:], d_wlin[:, g])

            # ---- persistent activation buffers ----
            NH = 2           # group-halves for elementwise batching
            GH = GPC // NH   # groups per half
            # hbuf split per group-half so step t+1's matmuls for half 0
            # don't serialize behind half 1's elementwise writes.
            hbuf = [
                [
                    apool.tile(
                        [128, GH, A, T], bf16, tag=f"hbuf{l}_{h}", name=f"hbuf{l}_{h}"
                    )
                    for h in range(NH)
                ]
                for l in range(2)
            ]
            fcbf = apool.tile([128, C2, GPC, T], bf16, tag="fcbf", name="fcbf")
            lin1bf = apool.tile([128, C2, GPC, T], bf16, tag="lin1bf", name="lin1bf")
            s_lin1 = apool.tile([128, C2, GPC, T], f32, tag="lin1", name="s_lin1")
            s_expl = apool.tile([128, C2, GPC, T], f32, tag="expl", name="s_expl")
            s_beta = apool.tile([128, C2, GPC, T], f32, tag="beta", name="s_beta")
            s_gamma = apool.tile([1, GPC, T], f32, tag="gamma", name="s_gamma")
            s_hn = apool.tile([128, 2, GPC, A], f32, tag="hn", name="s_hn")
            s_cn = apool.tile([128, 2, GPC, A], f32, tag="cn", name="s_cn")

            # ================= the two LSTM layers =================
            # Gate chunk order is host-permuted to [i,i,f,f,o,o,g,g] so the
            # sigmoid covers one contiguous [*, 0:6] slice (single ACT op).
            for l in range(2):
                # --- x-projection for all groups, all timesteps ---
                # xp[p, t, g, c] = (W_ih x_t)[perm c][128c+p] + bias
                xp = apool.tile(
                    [128, T, GPC, C8], f32, tag="xp", bufs=2, name=f"xp{l}"
                )
                for g in range(GPC):
                    for c in range(C8):
                        ps = pspool.tile(
                            [128, T], f32, tag="xp_ps", bufs=2, name=f"xps{l}_{g}_{c}"
                        )
                        for a in range(A):
                            if l == 0:
                                rhs = s_xT[:, g, a, :]
                            else:
                                rhs = hbuf[0][g // GH][:, g % GH, a, :]
                            nc.tensor.matmul(
                                ps[:],
                                w_ih[l][g][:, a, c, :],
                                rhs,
                                start=(a == 0),
                                stop=(a == 1),
                            )
                        nc.vector.tensor_scalar_add(
                            xp[:, :, g, c], ps[:], s_b[l][:, g, c : c + 1]
                        )

                # --- recurrence ---
                cst = [
                    apool.tile(
                        [128, GH, A], f32, tag=f"cst{l}_{h}", name=f"cst{l}_{h}"
                    )
                    for h in range(NH)
                ]
                for t in range(T):
                    pss = []
                    for h in range(NH):
                        g0 = h * GH
                        if t > 0:
                            ps = pspool.tile(
                                [128, GH, C8],
                                f32,
                                tag="gate_ps",
                                bufs=3,
                                name=f"gps{l}_{t}_{h}",
                            )
                            for gi in range(GH):
                                g = g0 + gi
                                for c in range(C8):
                                    for a in range(A):
                                        nc.tensor.matmul(
                                            ps[:, gi, c : c + 1],
                                            w_hh[l][g][:, a, c, :],
                                            hbuf[l][h][:, g % GH, a, t - 1 : t],
                                            start=(a == 0),
                                            stop=(a == 1),
                                        )
                            pss.append(ps)
                        else:
                            pss.append(None)

                    for h in range(NH):
                        g0 = h * GH
                        ps = pss[h]
                        if t > 0:
                            gb = tpool.tile(
                                [128, GH, C8], f32, tag="gb", bufs=3, name=f"gb{l}{t}{h}"
                            )
                            nc.vector.tensor_add(
                                gb[:], ps[:], xp[:, t, g0 : g0 + GH, :]
                            )
                            src = gb
                        else:
                            src = xp[:, 0, g0 : g0 + GH, :]
                        # sigmoid(i,f,o) in one op; tanh(g) in another
                        sif = tpool.tile(
                            [128, GH, 6], f32, tag="sif", bufs=3, name=f"sif{l}{t}{h}"
                        )
                        nc.scalar.activation(sif[:], src[:, :, 0:6], Sig)
                        tg = tpool.tile(
                            [128, GH, A], f32, tag="tg", bufs=3, name=f"tg{l}{t}{h}"
                        )
                        nc.scalar.activation(tg[:], src[:, :, 6:8], Tanh)

                        if t == 0:
                            # c = i * g
                            nc.vector.tensor_mul(cst[h][:], sif[:, :, 0:2], tg[:])
                        else:
                            fmul = tpool.tile(
                                [128, GH, A], f32, tag="fmul", bufs=3,
                                name=f"fm{l}{t}{h}",
                            )
                            nc.vector.tensor_mul(fmul[:], sif[:, :, 2:4], cst[h][:])
                            ig = tpool.tile(
                                [128, GH, A], f32, tag="ig", bufs=3, name=f"ig{l}{t}{h}"
                            )
                            nc.vector.tensor_mul(ig[:], sif[:, :, 0:2], tg[:])
                            nc.vector.tensor_add(cst[h][:], fmul[:], ig[:])
                        tch = tpool.tile(
                            [128, GH, A], f32, tag="tch", bufs=3, name=f"tc{l}{t}{h}"
                        )
                        nc.scalar.activation(tch[:], cst[h][:], Tanh)
                        # h_t (bf16) into the sequence buffer
                        nc.vector.tensor_mul(
                            hbuf[l][h][:, :, :, t], sif[:, :, 4:6], tch[:]
                        )
                        if t == T - 1:
                            nc.vector.tensor_mul(
                                s_hn[:, l, g0 : g0 + GH, :], sif[:, :, 4:6], tch[:]
                            )
                            nc.vector.tensor_copy(
                                s_cn[:, l, g0 : g0 + GH, :], cst[h][:]
                            )

            # ================= per-group Linear =================
            for g in range(GPC):
                ps = pspool.tile(
                    [128, C2, T], f32, tag="fc_ps", bufs=1, name=f"fcps{g}"
                )
                for c2 in range(C2):
                    for a in range(A):
                        nc.tensor.matmul(
                            ps[:, c2, :],
                            w_lin[g][:, a, c2, :],
                            hbuf[1][g // GH][:, g % GH, a, :],
                            start=(a == 0),
                            stop=(a == 1),
                        )
                for c2 in range(C2):
                    nc.vector.tensor_scalar_add(
                        fcbf[:, c2, g, :], ps[:, c2, :], s_blin[:, g, c2 : c2 + 1]
                    )

            # ================= shared heads (batched over groups) =========
            # softplus = ln(1 + exp(x)): all Exp ACTs first, then all Ln
            # ACTs, so the engine switches activation tables only once.
            betaexp = [None, None]
            for c2 in range(C2):
                psA = pspool.tile(
                    [128, GPC * T], f32, tag="head", bufs=2, name=f"psA{c2}"
                )
                for a in range(A):
                    nc.tensor.matmul(
                        psA[:],
                        s_w1t[:, a, c2, :],
                        fcbf[:, a].rearrange("p g t -> p (g t)"),
                        start=(a == 0),
                        stop=(a == 1),
                    )
                nc.vector.tensor_scalar_add(
                    s_lin1[:, c2].rearrange("p g t -> p (g t)"),
                    psA[:],
                    s_b1[:, c2 : c2 + 1],
                )
                nc.vector.tensor_scalar_add(
                    lin1bf[:, c2].rearrange("p g t -> p (g t)"),
                    psA[:],
                    s_b1[:, c2 : c2 + 1],
                )
                nc.scalar.activation(
                    s_expl[:, c2].rearrange("p g t -> p (g t)"),
                    psA[:],
                    Exp,
                    bias=s_b1[:, c2 : c2 + 1],
                )
                psB = pspool.tile(
                    [128, GPC * T], f32, tag="head", bufs=2, name=f"psB{c2}"
                )
                for a in range(A):
                    nc.tensor.matmul(
                        psB[:],
                        s_w2t[:, a, c2, :],
                        fcbf[:, a].rearrange("p g t -> p (g t)"),
                        start=(a == 0),
                        stop=(a == 1),
                    )
                if betaexp[0] is None:
                    betaexp[0] = tpool.tile(
                        [128, C2, GPC * T], f32, tag="betaexp", name="betaexp"
                    )
                nc.scalar.activation(
                    betaexp[0][:, c2, :], psB[:], Exp, bias=s_b2[:, c2 : c2 + 1]
                )
            # single Ln over both halves -> one activation-table switch
            nc.scalar.activation(
                s_beta.rearrange("p c g t -> p (c g t)"),
                betaexp[0].rearrange("p c n -> p (c n)"),
                Ln,
                bias=1.0,
            )

            psG = pspool.tile([1, GPC * T], f32, tag="head", bufs=2, name="psG")
            for a in range(A):
                nc.tensor.matmul(
                    psG[:],
                    s_wdt[:, a : a + 1],
                    lin1bf[:, a].rearrange("p g t -> p (g t)"),
                    start=(a == 0),
                    stop=(a == 1),
                )
            nc.vector.tensor_scalar_add(
                s_gamma.rearrange("p g t -> p (g t)"), psG[:], s_bd[:, 0:1]
            )

            # ---- outputs ----
            nc.sync.dma_start(d_lin1[:], s_lin1[:])
            nc.sync.dma_start(d_expl[:], s_expl[:])
            nc.sync.dma_start(d_beta[:], s_beta[:])
            nc.sync.dma_start(d_gamma[:], s_gamma[:])
            nc.sync.dma_start(d_hn[:], s_hn[:])
            nc.sync.dma_start(d_cn[:], s_cn[:])

    nc.compile()
    return nc


def _prep_core_inputs(core, data, Wih0, Whh0, bih0, bhh0, Wih1, Whh1, bih1, bhh1,
                      Wlin, blin, W1, b1, W2, b2, Wd, bd):
    """Host-side shard + retile + bf16 cast for one core."""
    lo, hi = core * GPC, (core + 1) * GPC
    # PyTorch gate order is (i,f,g,o) in 256-blocks = chunks [i,i,f,f,g,g,o,o];
    # the device wants [i,i,f,f,o,o,g,g] (contiguous sigmoid slice).
    GPERM = np.array([0, 1, 2, 3, 6, 7, 4, 5])

    def wtiles(W, cdim):
        # W: [GPC, cdim*128, 256] -> [128(q), GPC, A, cdim, 128(m)]
        arr = W.reshape(GPC, cdim, 128, A, 128)  # [g, c, m, a, q]
        if cdim == C8:
            arr = arr[:, GPERM]
        return np.ascontiguousarray(arr.transpose(4, 0, 3, 1, 2)).astype(BF16)

    def btiles(b, cdim):
        # b: [GPC, cdim*128] -> [128(p), GPC, cdim]
        arr = b.reshape(GPC, cdim, 128)
        if cdim == C8:
            arr = arr[:, GPERM]
        return np.ascontiguousarray(arr.transpose(2, 0, 1)).astype(np.float32)

    d = {}
    x = data[:, lo:hi, :]  # [T, GPC, 256]
    d["xT"] = np.ascontiguousarray(
        x.reshape(T, GPC, A, 128).transpose(3, 1, 2, 0)
    ).astype(BF16)
    d["wih0"] = wtiles(Wih0[lo:hi], C8)
    d["whh0"] = wtiles(Whh0[lo:hi], C8)
    d["wih1"] = wtiles(Wih1[lo:hi], C8)
    d["whh1"] = wtiles(Whh1[lo:hi], C8)
    d["wlin"] = wtiles(Wlin[lo:hi], C2)
    d["b0"] = btiles(bih0[lo:hi] + bhh0[lo:hi], C8)
    d["b1c"] = btiles(bih1[lo:hi] + bhh1[lo:hi], C8)
    d["blin"] = btiles(blin[lo:hi], C2)
    # shared weights: [C2*128, 256] -> [128(q), A, C2, 128(m)]
    for name, W in (("w1t", W1), ("w2t", W2)):
        arr = W.reshape(C2, 128, A, 128)  # [c2, m, a, q]
        d[name] = np.ascontiguousarray(arr.transpose(3, 2, 0, 1)).astype(BF16)
    d["wdt"] = np.ascontiguousarray(Wd.reshape(A, 128).T).astype(BF16)
    d["b1"] = np.ascontiguousarray(b1.reshape(C2, 128).T).astype(np.float32)
    d["b2"] = np.ascontiguousarray(b2.reshape(C2, 128).T).astype(np.float32)
    d["bd"] = bd.reshape(1, 1).astype(np.float32)
    return d


def _get_compiled():
    if "nc" not in _COMPILED:
        _COMPILED["nc"] = _build_nc()
    return _COMPILED["nc"]


def run_device(in_maps, trace=False, tmpdir=None):
    from concourse import bass_utils

    nc = _get_compiled()
    kw = {}
    if trace:
        kw = dict(trace=True, tmpdir=tmpdir)
    res = bass_utils.run_bass_kernel_spmd(
        nc, in_maps, core_ids=list(range(NCORES)), **kw
    )
    return res


def assemble(results):
    """Per-core device outputs -> full reference-shaped outputs."""
    lin1 = np.empty((G, T, H), np.float32)
    expl = np.empty((G, T, H), np.float32)
    beta = np.empty((G, T, H), np.float32)
    gamma = np.empty((G, T, 1), np.float32)
    hN = np.empty((G, 2, H), np.float32)
    cN = np.empty((G, 2, H), np.float32)
    for core in range(NCORES):
        r = results[core]
        lo = core * GPC
        # [128(p), C2, GPC, T] -> [g, t, c2*128+p]
        for name, dst in (("lin1o", lin1), ("explo", expl), ("betao", beta)):
            v = r[name].reshape(128, C2, GPC, T)
            dst[lo : lo + GPC] = v.transpose(2, 3, 1, 0).reshape(GPC, T, H)
        gamma[lo : lo + GPC] = r["gammao"].reshape(GPC, T, 1)
        # [128(p), l, g, a] -> [g, l, a*128+p]
        for name, dst in (("hno", hN), ("cno", cN)):
            v = r[name].reshape(128, 2, GPC, A)
            dst[lo : lo + GPC] = v.transpose(2, 1, 3, 0).reshape(GPC, 2, H)
    delta = expl / expl.sum(axis=0, keepdims=True)
    return gamma, beta, delta, hN, cN


def kernel(**inputs):
    in_maps = [_prep_core_inputs(c, **inputs) for c in range(NCORES)]
    res = run_device(in_maps)
    return assemble(res.results)


# revision 13
# speedup vs baseline: 1.3434x; 1.1110x over previous
"""Bass/Trainium2 kernel for nn_Encoder_32452772888844.

64 independent 2-layer LSTM(256) encoders + per-group Linear(256,256),
then shared heads:
  lin1  = fc @ W1.T + b1
  delta = softmax(lin1, axis=0)   (over the 64 groups)
  beta  = softplus(fc @ W2.T + b2)
  gamma = lin1 @ Wd.T + bd
Sharding: pure group parallelism — 8 groups per NeuronCore; each core
computes everything for its groups including exp(lin1); the softmax
normalization (a sum over the 64-group axis) is applied on the host.

Device-side formulation (per core):
  - All matmuls are weight-stationary: lhsT tiles [K=128, M=128] are
    (transposed) weight blocks, the moving operand is the activation
    vector/sequence. Gate results land in PSUM with the hidden dim on
    partitions, which makes the LSTM cell elementwise work efficient.
  - Weights are cast to bf16 on the host (PSUM accumulates in fp32).
  - x-projections (with biases) for all 10 timesteps are batched up
    front per layer; at each recurrence step an identity matmul
    preloads x-projection+bias into the gates PSUM region and the
    h-recurrence matmuls accumulate on top, so the ScalarE activations
    read gates straight out of PSUM (no elementwise add on the
    critical path).
  - Groups are processed in two halves per step so one half's cell
    math overlaps the other half's matmuls.
  - Gate chunk order is host-permuted from PyTorch's (i,f,g,o) to
    (i,i,f,f,o,o,g,g) 128-blocks so sigmoid covers one contiguous
    slice.
"""

import numpy as np
import ml_dtypes

T = 10
IN = 256
H = 256
G = 64
NCORES = 8
GPC = G // NCORES  # groups per core
A = 2   # 128-halves of 256
C8 = 8  # 128-chunks of 1024
C2 = 2  # 128-chunks of 256

BF16 = ml_dtypes.bfloat16

_COMPILED = {}


def _build_nc():
    import concourse.tile as tile
    from concourse import bacc, mybir

    f32 = mybir.dt.float32
    bf16 = mybir.dt.bfloat16
    Sig = mybir.ActivationFunctionType.Sigmoid
    Tanh = mybir.ActivationFunctionType.Tanh
    Exp = mybir.ActivationFunctionType.Exp
    Ln = mybir.ActivationFunctionType.Ln

    nc = bacc.Bacc(None, target_bir_lowering=False)

    # ---- DRAM parameters (per-core shards, host-prepared layouts) ----
    d_xT = nc.dram_tensor("xT", [128, GPC, A, T], bf16, kind="ExternalInput")
    d_ident = nc.dram_tensor("ident", [128, 128], bf16, kind="ExternalInput")
    # wih and whh merged: dim1 -> 0=ih, 1=hh
    d_w = [
        nc.dram_tensor("w0", [128, GPC, 2, A, C8, 128], bf16, kind="ExternalInput"),
        nc.dram_tensor("w1", [128, GPC, 2, A, C8, 128], bf16, kind="ExternalInput"),
    ]
    d_wlin = nc.dram_tensor("wlin", [128, GPC, A, C2, 128], bf16, kind="ExternalInput")
    d_w1t = nc.dram_tensor("w1t", [128, A, C2, 128], bf16, kind="ExternalInput")
    d_w2t = nc.dram_tensor("w2t", [128, A, C2, 128], bf16, kind="ExternalInput")
    d_wdt = nc.dram_tensor("wdt", [128, A], bf16, kind="ExternalInput")
    # combined lstm biases, broadcast over T on the host
    d_b = [
        nc.dram_tensor("b0", [128, GPC, C8, T], f32, kind="ExternalInput"),
        nc.dram_tensor("b1c", [128, GPC, C8, T], f32, kind="ExternalInput"),
    ]
    d_blin = nc.dram_tensor("blin", [128, GPC, C2], f32, kind="ExternalInput")
    d_b1 = nc.dram_tensor("b1", [128, C2], f32, kind="ExternalInput")
    d_b2 = nc.dram_tensor("b2", [128, C2], f32, kind="ExternalInput")
    d_bd = nc.dram_tensor("bd", [1, 1], f32, kind="ExternalInput")

    d_lin1 = nc.dram_tensor("lin1o", [128, C2, GPC, T], f32, kind="ExternalOutput")
    d_expl = nc.dram_tensor("explo", [128, C2, GPC, T], f32, kind="ExternalOutput")
    d_beta = nc.dram_tensor("betao", [128, C2, GPC, T], f32, kind="ExternalOutput")
    d_gamma = nc.dram_tensor("gammao", [1, GPC, T], f32, kind="ExternalOutput")
    d_hn = nc.dram_tensor("hno", [128, 2, GPC, A], f32, kind="ExternalOutput")
    d_cn = nc.dram_tensor("cno", [128, 2, GPC, A], f32, kind="ExternalOutput")

    with tile.TileContext(nc) as tc:
        with (
            tc.tile_pool(name="wpool", bufs=1) as wpool,
            tc.tile_pool(name="apool", bufs=1) as apool,
            tc.tile_pool(name="tpool", bufs=1) as tpool,
            tc.tile_pool(name="pspool", bufs=1, space="PSUM") as pspool,
        ):
            # ---- loads, in compute-consumption order ----
            s_xT = apool.tile([128, GPC, A, T], bf16, tag="xT", name="s_xT")
            nc.sync.dma_start(s_xT[:], d_xT[:])
            s_ident = apool.tile([128, 128], bf16, tag="ident", name="s_ident")
            nc.sync.dma_start(s_ident[:], d_ident[:])
            s_b = []
            for l in range(2):
                t_ = apool.tile([128, GPC, C8, T], f32, tag=f"b{l}", name=f"s_b{l}")
                nc.sync.dma_start(t_[:], d_b[l][:])
                s_b.append(t_)
            s_blin = apool.tile([128, GPC, C2], f32, tag="blin", name="s_blin")
            nc.sync.dma_start(s_blin[:], d_blin[:])
            s_b1 = apool.tile([128, C2], f32, tag="b1", name="s_b1")
            nc.sync.dma_start(s_b1[:], d_b1[:])
            s_b2 = apool.tile([128, C2], f32, tag="b2", name="s_b2")
            nc.sync.dma_start(s_b2[:], d_b2[:])
            s_bd = apool.tile([1, 1], f32, tag="bd", name="s_bd")
            nc.sync.dma_start(s_bd[:], d_bd[:])
            s_w1t = apool.tile([128, A, C2, 128], bf16, tag="w1t", name="s_w1t")
            nc.sync.dma_start(s_w1t[:], d_w1t[:])
            s_w2t = apool.tile([128, A, C2, 128], bf16, tag="w2t", name="s_w2t")
            nc.sync.dma_start(s_w2t[:], d_w2t[:])
            s_wdt = apool.tile([128, A], bf16, tag="wdt", name="s_wdt")
            nc.sync.dma_start(s_wdt[:], d_wdt[:])

            # per-(layer, group) weight tiles so compute on group g only
            # waits on g's own DMA; layer-0 weights stream in first
            w_lg = [[None] * GPC for _ in range(2)]
            w_lin = [None] * GPC
            for l in range(2):
                for g in range(GPC):
                    w_lg[l][g] = wpool.tile(
                        [128, 2, A, C8, 128], bf16, tag=f"w{l}_{g}", name=f"w{l}_{g}"
                    )
                    nc.sync.dma_start(w_lg[l][g][:], d_w[l][:, g])
            for g in range(GPC):
                w_lin[g] = wpool.tile(
                    [128, A, C2, 128], bf16, tag=f"wlin_{g}", name=f"wlin_{g}"
                )
                nc.sync.dma_start(w_lin[g][:], d_wlin[:, g])

            # ---- persistent activation buffers ----
            NH = 2           # group-halves for elementwise batching
            GH = GPC // NH   # groups per half
            hbuf = [
                [
                    apool.tile(
                        [128, GH, A, T], bf16, tag=f"hbuf{l}_{h}", name=f"hbuf{l}_{h}"
                    )
                    for h in range(NH)
                ]
                for l in range(2)
            ]
            fcbf = apool.tile([128, C2, GPC, T], bf16, tag="fcbf", name="fcbf")
            lin1bf = apool.tile([128, C2, GPC, T], bf16, tag="lin1bf", name="lin1bf")
            s_lin1 = apool.tile([128, C2, GPC, T], f32, tag="lin1", name="s_lin1")
            s_expl = apool.tile([128, C2, GPC, T], f32, tag="expl", name="s_expl")
            s_beta = apool.tile([128, C2, GPC, T], f32, tag="beta", name="s_beta")
            s_gamma = apool.tile([1, GPC, T], f32, tag="gamma", name="s_gamma")
            s_hn = apool.tile([128, 2, GPC, A], f32, tag="hn", name="s_hn")
            s_cn = apool.tile([128, 2, GPC, A], f32, tag="cn", name="s_cn")

            # ================= the two LSTM layers =================
            for l in range(2):
                # --- x-projection+bias for all groups/timesteps (bf16) ---
                # xp[p, t, g, c] = (W_ih x_t + bih + bhh)[perm c][128c+p]
                xp = apool.tile(
                    [128, T, GPC, C8], bf16, tag="xp", bufs=2, name=f"xp{l}"
                )
                for g in range(GPC):
                    ps = pspool.tile(
                        [128, C8, T], f32, tag="xp_ps", bufs=2, name=f"xps{l}_{g}"
                    )
                    for c in range(C8):
                        for a in range(A):
                            if l == 0:
                                rhs = s_xT[:, g, a, :]
                            else:
                                rhs = hbuf[0][g // GH][:, g % GH, a, :]
                            nc.tensor.matmul(
                                ps[:, c, :],
                                w_lg[l][g][:, 0, a, c, :],
                                rhs,
                                start=(a == 0),
                                stop=(a == 1),
                            )
                    # single bias add per group; bias pre-broadcast over T
                    nc.vector.tensor_add(
                        xp[:, :, g, :].rearrange("p t c -> p c t"),
                        ps[:],
                        s_b[l][:, g],
                    )

                # --- recurrence ---
                cst = [
                    apool.tile(
                        [128, GH, A], f32, tag=f"cst{l}_{h}", name=f"cst{l}_{h}"
                    )
                    for h in range(NH)
                ]
                for t in range(T):
                    pss = []
                    for h in range(NH):
                        g0 = h * GH
                        ps = pspool.tile(
                            [128, GH, C8], f32, tag="gate_ps", bufs=3,
                            name=f"gps{l}_{t}_{h}",
                        )
                        # preload xp(+bias) into the gates PSUM region.
                        # stop=True closes the sim's accumulation group; the
                        # recurrence matmuls below keep accumulating on top
                        # (hardware per-element has_written semantics) with
                        # skip_group_check.
                        nc.tensor.matmul(
                            ps[:],
                            s_ident[:],
                            xp[:, t, g0 : g0 + GH, :],
                            start=True,
                            stop=True,
                        )
                        if t > 0:
                            for gi in range(GH):
                                g = g0 + gi
                                for c in range(C8):
                                    for a in range(A):
                                        nc.tensor.matmul(
                                            ps[:, gi, c : c + 1],
                                            w_lg[l][g][:, 1, a, c, :],
                                            hbuf[l][h][:, g % GH, a, t - 1 : t],
                                            start=False,
                                            stop=(a == 1),
                                            skip_group_check=True,
                                        )
                        pss.append(ps)

                    for h in range(NH):
                        g0 = h * GH
                        ps = pss[h]
                        # gates are complete in PSUM; ACT reads PSUM direct
                        sif = tpool.tile(
                            [128, GH, 6], f32, tag="sif", bufs=3, name=f"sif{l}{t}{h}"
                        )
                        nc.scalar.activation(sif[:], ps[:, :, 0:6], Sig)
                        tg = tpool.tile(
                            [128, GH, A], f32, tag="tg", bufs=3, name=f"tg{l}{t}{h}"
                        )
                        nc.scalar.activation(tg[:], ps[:, :, 6:8], Tanh)

                        if t == 0:
                            # c = i * g
                            nc.vector.tensor_mul(cst[h][:], sif[:, :, 0:2], tg[:])
                        else:
                            fmul = tpool.tile(
                                [128, GH, A], f32, tag="fmul", bufs=3,
                                name=f"fm{l}{t}{h}",
                            )
                            nc.vector.tensor_mul(fmul[:], sif[:, :, 2:4], cst[h][:])
                            ig = tpool.tile(
                                [128, GH, A], f32, tag="ig", bufs=3, name=f"ig{l}{t}{h}"
                            )
                            nc.vector.tensor_mul(ig[:], sif[:, :, 0:2], tg[:])
                            nc.vector.tensor_add(cst[h][:], fmul[:], ig[:])
                        tch = tpool.tile(
                            [128, GH, A], f32, tag="tch", bufs=3, name=f"tc{l}{t}{h}"
                        )
                        nc.scalar.activation(tch[:], cst[h][:], Tanh)
                        # h_t (bf16) into the sequence buffer
                        nc.vector.tensor_mul(
                            hbuf[l][h][:, :, :, t], sif[:, :, 4:6], tch[:]
                        )
                        if t == T - 1:
                            nc.vector.tensor_mul(
                                s_hn[:, l, g0 : g0 + GH, :], sif[:, :, 4:6], tch[:]
                            )
                            nc.vector.tensor_copy(
                                s_cn[:, l, g0 : g0 + GH, :], cst[h][:]
                            )

            # ================= per-group Linear =================
            for g in range(GPC):
                ps = pspool.tile(
                    [128, C2, T], f32, tag="fc_ps", bufs=1, name=f"fcps{g}"
                )
                for c2 in range(C2):
                    for a in range(A):
                        nc.tensor.matmul(
                            ps[:, c2, :],
                            w_lin[g][:, a, c2, :],
                            hbuf[1][g // GH][:, g % GH, a, :],
                            start=(a == 0),
                            stop=(a == 1),
                        )
                for c2 in range(C2):
                    nc.vector.tensor_scalar_add(
                        fcbf[:, c2, g, :], ps[:, c2, :], s_blin[:, g, c2 : c2 + 1]
                    )

            # ================= shared heads (batched over groups) =========
            # softplus = ln(1 + exp(x)): all Exp ACTs, then one Ln ACT,
            # so the scalar engine switches activation tables only once.
            betaexp = None
            for c2 in range(C2):
                psA = pspool.tile(
                    [128, GPC * T], f32, tag="head", bufs=2, name=f"psA{c2}"
                )
                for a in range(A):
                    nc.tensor.matmul(
                        psA[:],
                        s_w1t[:, a, c2, :],
                        fcbf[:, a].rearrange("p g t -> p (g t)"),
                        start=(a == 0),
                        stop=(a == 1),
                    )
                nc.vector.tensor_scalar_add(
                    s_lin1[:, c2].rearrange("p g t -> p (g t)"),
                    psA[:],
                    s_b1[:, c2 : c2 + 1],
                )
                nc.vector.tensor_scalar_add(
                    lin1bf[:, c2].rearrange("p g t -> p (g t)"),
                    psA[:],
                    s_b1[:, c2 : c2 + 1],
                )
                nc.scalar.activation(
                    s_expl[:, c2].rearrange("p g t -> p (g t)"),
                    psA[:],
                    Exp,
                    bias=s_b1[:, c2 : c2 + 1],
                )
                psB = pspool.tile(
                    [128, GPC * T], f32, tag="head", bufs=2, name=f"psB{c2}"
                )
                for a in range(A):
                    nc.tensor.matmul(
                        psB[:],
                        s_w2t[:, a, c2, :],
                        fcbf[:, a].rearrange("p g t -> p (g t)"),
                        start=(a == 0),
                        stop=(a == 1),
                    )
                if betaexp is None:
                    betaexp = tpool.tile(
                        [128, C2, GPC * T], f32, tag="betaexp", name="betaexp"
                    )
                nc.scalar.activation(
                    betaexp[:, c2, :], psB[:], Exp, bias=s_b2[:, c2 : c2 + 1]
                )
            nc.scalar.activation(
                s_beta.rearrange("p c g t -> p (c g t)"),
                betaexp.rearrange("p c n -> p (c n)"),
                Ln,
                bias=1.0,
            )

            psG = pspool.tile([1, GPC * T], f32, tag="head", bufs=2, name="psG")
            for a in range(A):
                nc.tensor.matmul(
                    psG[:],
                    s_wdt[:, a : a + 1],
                    lin1bf[:, a].rearrange("p g t -> p (g t)"),
                    start=(a == 0),
                    stop=(a == 1),
                )
            nc.vector.tensor_scalar_add(
                s_gamma.rearrange("p g t -> p (g t)"), psG[:], s_bd[:, 0:1]
            )

            # ---- outputs ----
            nc.sync.dma_start(d_lin1[:], s_lin1[:])
            nc.sync.dma_start(d_expl[:], s_expl[:])
            nc.sync.dma_start(d_beta[:], s_beta[:])
            nc.sync.dma_start(d_gamma[:], s_gamma[:])
            nc.sync.dma_start(d_hn[:], s_hn[:])
            nc.sync.dma_start(d_cn[:], s_cn[:])

    nc.compile()
    return nc


def _prep_core_inputs(core, data, Wih0, Whh0, bih0, bhh0, Wih1, Whh1, bih1, bhh1,
                      Wlin, blin, W1, b1, W2, b2, Wd, bd):
    """Host-side shard + retile + bf16 cast for one core."""
    lo, hi = core * GPC, (core + 1) * GPC
    # PyTorch gate order is (i,f,g,o) in 256-blocks = chunks [i,i,f,f,g,g,o,o];
    # the device wants [i,i,f,f,o,o,g,g] (contiguous sigmoid slice).
    GPERM = np.array([0, 1, 2, 3, 6, 7, 4, 5])

    def wtiles(W, cdim):
        # W: [GPC, cdim*128, 256] -> [128(q), GPC, A, cdim, 128(m)]
        arr = W.reshape(GPC, cdim, 128, A, 128)  # [g, c, m, a, q]
        if cdim == C8:
            arr = arr[:, GPERM]
        return np.ascontiguousarray(arr.transpose(4, 0, 3, 1, 2)).astype(BF16)

    def btiles(b, cdim):
        # b: [GPC, cdim*128] -> [128(p), GPC, cdim]
        arr = b.reshape(GPC, cdim, 128)
        if cdim == C8:
            arr = arr[:, GPERM]
        return np.ascontiguousarray(arr.transpose(2, 0, 1)).astype(np.float32)

    d = {}
    x = data[:, lo:hi, :]  # [T, GPC, 256]
    d["xT"] = np.ascontiguousarray(
        x.reshape(T, GPC, A, 128).transpose(3, 1, 2, 0)
    ).astype(BF16)
    d["ident"] = np.eye(128, dtype=BF16)
    d["w0"] = np.stack([wtiles(Wih0[lo:hi], C8), wtiles(Whh0[lo:hi], C8)], axis=2)
    d["w1"] = np.stack([wtiles(Wih1[lo:hi], C8), wtiles(Whh1[lo:hi], C8)], axis=2)
    d["wlin"] = wtiles(Wlin[lo:hi], C2)
    # biases broadcast over T so one DVE add applies them per group
    d["b0"] = np.ascontiguousarray(
        np.broadcast_to(
            btiles(bih0[lo:hi] + bhh0[lo:hi], C8)[..., None], (128, GPC, C8, T)
        )
    )
    d["b1c"] = np.ascontiguousarray(
        np.broadcast_to(
            btiles(bih1[lo:hi] + bhh1[lo:hi], C8)[..., None], (128, GPC, C8, T)
        )
    )
    d["blin"] = btiles(blin[lo:hi], C2)
    # shared weights: [C2*128, 256] -> [128(q), A, C2, 128(m)]
    for name, W in (("w1t", W1), ("w2t", W2)):
        arr = W.reshape(C2, 128, A, 128)  # [c2, m, a, q]
        d[name] = np.ascontiguousarray(arr.transpose(3, 2, 0, 1)).astype(BF16)
    d["wdt"] = np.ascontiguousarray(Wd.reshape(A, 128).T).astype(BF16)
    d["b1"] = np.ascontiguousarray(b1.reshape(C2, 128).T).astype(np.float32)
    d["b2"] = np.ascontiguousarray(b2.reshape(C2, 128).T).astype(np.float32)
    d["bd"] = bd.reshape(1, 1).astype(np.float32)
    return d


def _get_compiled():
    if "nc" not in _COMPILED:
        _COMPILED["nc"] = _build_nc()
    return _COMPILED["nc"]


def run_device(in_maps, trace=False, tmpdir=None):
    from concourse import bass_utils

    nc = _get_compiled()
    kw = {}
    if trace:
        kw = dict(trace=True, tmpdir=tmpdir)
    res = bass_utils.run_bass_kernel_spmd(
        nc, in_maps, core_ids=list(range(NCORES)), **kw
    )
    return res


def assemble(results):
    """Per-core device outputs -> full reference-shaped outputs."""
    lin1 = np.empty((G, T, H), np.float32)
    expl = np.empty((G, T, H), np.float32)
    beta = np.empty((G, T, H), np.float32)
    gamma = np.empty((G, T, 1), np.float32)
    hN = np.empty((G, 2, H), np.float32)
    cN = np.empty((G, 2, H), np.float32)
    for core in range(NCORES):
        r = results[core]
        lo = core * GPC
        # [128(p), C2, GPC, T] -> [g, t, c2*128+p]
        for name, dst in (("lin1o", lin1), ("explo", expl), ("betao", beta)):
            v = r[name].reshape(128, C2, GPC, T)
            dst[lo : lo + GPC] = v.transpose(2, 3, 1, 0).reshape(GPC, T, H)
        gamma[lo : lo + GPC] = r["gammao"].reshape(GPC, T, 1)
        # [128(p), l, g, a] -> [g, l, a*128+p]
        for name, dst in (("hno", hN), ("cno", cN)):
            v = r[name].reshape(128, 2, GPC, A)
            dst[lo : lo + GPC] = v.transpose(2, 1, 3, 0).reshape(GPC, 2, H)
    delta = expl / expl.sum(axis=0, keepdims=True)
    return gamma, beta, delta, hN, cN


def kernel(**inputs):
    in_maps = [_prep_core_inputs(c, **inputs) for c in range(NCORES)]
    res = run_device(in_maps)
    return assemble(res.results)
